# revision 1
# baseline (speedup 1.0000x reference)
"""Trainium2 Bass kernel for nn_CNNBlock (proj_in -> scatter -> 2x dilated
conv+syncBN+relu+residual -> gather -> proj_out -> residual -> LayerNorm).

Data-parallel over the batch on 8 NeuronCores; BN batch stats synchronized
with a tiny AllGather per conv layer.

Self-contained: hardcodes shapes from the problem spec.
"""
import numpy as np

B = 128          # batch
NCORES = 8
BL = B // NCORES  # 16 images per core
S = 624          # stabilizers = 24*26 grid cells, row-major
H = 256          # hidden
C = 128          # conv channels
G = 26           # grid size
CH = 338         # conv output chunk = 13 rows * 26 cols
HALF = 312       # proj_in chunk = 12 rows * 26 cols
SL = BL * S      # 9984 rows per core
EPS = 1e-5
N_BN = B * G * G  # BN stat count per channel (full batch)
# Guarded grid layout: each of the 26 grid rows is 28 wide (cols 26,27 are
# zero guards) plus a 2-element leading zero guard, so every conv tap reads a
# 26-wide (even) window and out-of-row accesses land on zeros.
GW = 28
GOFF = 2
GBUF = GOFF + GW * G + 2  # 732

_CACHE = {}


def _build(ln_affine):
    import concourse.bacc as bacc
    import concourse.tile as tile
    from concourse import mybir

    F32 = mybir.dt.float32
    F32R = mybir.dt.float32r
    AF = mybir.ActivationFunctionType
    ALU = mybir.AluOpType
    AX = mybir.AxisListType

    nc = bacc.Bacc("TRN2", target_bir_lowering=False, debug=False,
                   enable_asserts=True, num_devices=NCORES)

    def din(name, shape):
        return nc.dram_tensor(name, shape, F32, kind="ExternalInput").ap()

    xT = din("xT", [H, SL])          # x shard transposed: [h, img*624+s]
    xR = din("xR", [SL, H])          # x shard + b_out, s-major
    w_in = din("w_in", [H, C])
    w0 = din("w0", [9, C, C])        # conv0 weights [tap][c_in][c_out]
    w1 = din("w1", [9, C, C])
    w_out = din("w_out", [C, H])
    bvec = din("bvec", [C, 8])       # cols: b_in, bn_g0, bn_b0, bn_g1, bn_b1
    if ln_affine:
        lng = din("lng", [C, H])     # ln_g broadcast over partitions
        lnb = din("lnb", [C, H])
    out = nc.dram_tensor("out", [SL, H], F32, kind="ExternalOutput").ap()

    with tile.TileContext(nc) as tc:
        with (
            tc.tile_pool(name="const", bufs=1) as const,
            tc.tile_pool(name="grids", bufs=1) as grids,
            tc.tile_pool(name="stream", bufs=6) as stream,
            tc.tile_pool(name="work", bufs=3) as work,
            tc.tile_pool(name="stats", bufs=1) as stats,
            tc.tile_pool(name="psum", bufs=8, space="PSUM") as psum,
            tc.tile_pool(name="dramp", bufs=4, space="DRAM") as dramp,
        ):
            # ---- constants ----
            bv = const.tile([C, 8], F32, name="bv")
            nc.sync.dma_start(bv[:], bvec[:, :])
            wi = []
            for k in range(2):
                w = const.tile([C, C], F32R, name=f"wi{k}")
                nc.sync.dma_start(w[:], w_in[k * C:(k + 1) * C, :].bitcast(F32R))
                wi.append(w)
            if ln_affine:
                lng_t = const.tile([C, H], F32, name="lng_t")
                nc.sync.dma_start(lng_t[:], lng[:, :])
                lnb_t = const.tile([C, H], F32, name="lnb_t")
                nc.sync.dma_start(lnb_t[:], lnb[:, :])
            zf = const.tile([C, 64], F32, name="zf")
            nc.vector.memset(zf[:], 0.0)
            zr = const.tile([C, 64], F32R, name="zr")
            nc.vector.tensor_copy(zr[:], zf[:])

            # Startup barrier: a tiny AllGather issued first absorbs the
            # cross-core launch skew while the front of the kernel (DMAs,
            # proj_in) runs, so the first real sync-BN AllGather doesn't
            # pay it. Collectives execute in order on the TOPSP queue.
            bar_in = dramp.tile([C, 2], F32, name="bar_in")
            nc.gpsimd.dma_start(bar_in[:], zf[:, 0:2])
            bar_out = dramp.tile([NCORES * C, 2], F32, name="bar_out")
            nc.gpsimd.collective_compute(
                "AllGather", ALU.bypass,
                replica_groups=[list(range(NCORES))],
                ins=[bar_in.opt()], outs=[bar_out.opt()])
            bar_sb = const.tile([C, 2], F32, name="bar_sb")
            nc.gpsimd.dma_start(bar_sb[:], bar_out[0:C, :])
            # eps_t = barrier*0 + EPS ties the barrier into the dataflow
            eps_t = const.tile([C, 1], F32, name="eps_t")
            nc.vector.tensor_scalar(eps_t[:], bar_sb[:, 0:1], 0.0, EPS,
                                    ALU.mult, ALU.add)

            # ---- persistent per-image buffers ----
            g0 = [grids.tile([C, GBUF], F32R, name=f"g0_{i}") for i in range(BL)]
            g1 = [grids.tile([C, GBUF], F32R, name=f"g1_{i}") for i in range(BL)]
            yb = [grids.tile([C, G * G], F32, name=f"y_{i}") for i in range(BL)]

            def gview(t):
                # [C, 26, 28] row view of a guarded grid buffer
                return t[:, GOFF:GOFF + G * GW].rearrange(
                    "p (r c) -> p r c", r=G, c=GW)

            # zero the guard elements (they persist for the whole kernel)
            for i in range(BL):
                for t, empty_rows in ((g0[i], True), (g1[i], False)):
                    nc.vector.tensor_copy(t[:, 0:GOFF], zr[:, 0:GOFF])
                    nc.vector.tensor_copy(
                        gview(t)[:, :, G:GW],
                        zr[:, 0:2 * G].rearrange("p (r c) -> p r c", r=G, c=2))
                    if empty_rows:
                        # grid rows 24-25 hold no stabilizers -> zeros
                        nc.vector.tensor_copy(
                            t[:, GOFF + 24 * GW:GOFF + 26 * GW],
                            zr[:, 0:2 * GW])

            sqs0 = stats.tile([C, 192], F32, name="sqs0")
            sqs1 = stats.tile([C, 192], F32, name="sqs1")
            zst = [stats.tile([C, 10], F32, name=f"zst_{i}") for i in range(BL)]
            for i in range(BL):
                nc.vector.memset(zst[i][:], 1.0)

            # ================= Stage A: proj_in =================
            for i in range(BL):
                for q in range(2):
                    o = i * S + q * HALF
                    xk0 = stream.tile([C, HALF], F32R, name="xk0", tag="xk0")
                    nc.sync.dma_start(xk0[:], xT[0:C, o:o + HALF].bitcast(F32R))
                    xk1 = stream.tile([C, HALF], F32R, name="xk1", tag="xk1")
                    nc.sync.dma_start(xk1[:], xT[C:H, o:o + HALF].bitcast(F32R))
                    pa = psum.tile([C, CH], F32, tag="ps", name=f"pa{i}_{q}")
                    nc.tensor.matmul(pa[:, 0:HALF], wi[0][:], xk0[:],
                                     start=True, stop=False)
                    nc.tensor.matmul(pa[:, 0:HALF], wi[1][:], xk1[:],
                                     start=False, stop=True)
                    # 312 = 12 grid rows of 26, rows 12q..12q+11
                    dst = gview(g0[i])[:, 12 * q:12 * q + 12, 0:G]
                    nc.scalar.activation(
                        dst, pa[:, 0:HALF].rearrange("p (r c) -> p r c",
                                                     r=12, c=G),
                        AF.Identity, bias=bv[:, 0:1], scale=1.0)

            # deferred weight loads: keep the startup DMA queues free for
            # the proj_in activation stream
            wt0, wt1 = [], []
            for t in range(9):
                w = const.tile([C, C], F32R, name=f"w0_{t}")
                nc.sync.dma_start(w[:], w0[t, :, :].bitcast(F32R))
                wt0.append(w)
            for t in range(9):
                w = const.tile([C, C], F32R, name=f"w1_{t}")
                nc.sync.dma_start(w[:], w1[t, :, :].bitcast(F32R))
                wt1.append(w)
            wo = const.tile([C, H], F32R, name="wo")
            nc.sync.dma_start(wo[:], w_out[:, :].bitcast(F32R))

            # ================= conv + stats helper =================
            def conv(src, wt, dil, sqs):
                # groups of 4 chunks (2 images): only half the PSUM banks per
                # group, so the next group's matmuls stream while this group's
                # PSUM->SBUF copies drain (keeps the PE HAM-warm)
                for grp in range(8):
                    psc = [psum.tile([C, CH], F32, tag="ps",
                                     name=f"pc{dil}_{grp}_{m}") for m in range(4)]
                    for t9 in range(9):
                        di = (t9 // 3 - 1) * dil
                        dj = (t9 % 3 - 1) * dil
                        for m in range(4):
                            img = grp * 2 + m // 2
                            q = m % 2
                            # clip rows so i+di stays in [0, 26); cols always
                            # span the full 26 (out-of-row cols hit guards=0)
                            r_lo = max(13 * q, -di)
                            r_hi = min(13 * q + 13, G - di)
                            nr = r_hi - r_lo
                            base = GOFF + (r_lo + di) * GW + dj
                            rhs = src[img][:, base:base + nr * GW].rearrange(
                                "p (r c) -> p r c", r=nr, c=GW)[:, :, 0:G]
                            oap = psc[m][:, (r_lo - 13 * q) * G:
                                          (r_hi - 13 * q) * G]
                            nc.tensor.matmul(oap, wt[t9][:], rhs,
                                             start=(t9 == 0), stop=(t9 == 8))
                    for m in range(4):
                        img = grp * 2 + m // 2
                        q = m % 2
                        cid = img * 2 + q
                        ysl = yb[img][:, q * CH:(q + 1) * CH]
                        nc.vector.tensor_copy(ysl, psc[m][:])
                        # per-chunk (count, mean, M2) even/odd triplets; both
                        # the sum and sumsq are reassembled in bn_sync
                        nc.vector.bn_stats(sqs[:, cid * 6:(cid + 1) * 6], ysl)

            # ============ sync-BN stats: AllGather + combine ============
            def bn_sync(sqs, gcol, bcol, tag):
                Sl = stats.tile([C, 2], F32, name=f"Sl{tag}")
                # sum = 169 * sum(mean_e + mean_o) over the 32 chunks;
                # sumsq = sum(M2_e + M2_o) + 169 * sum(mean_e^2 + mean_o^2)
                v = sqs[:].rearrange("p (k t) -> p t k", k=32, t=6)
                me2 = stats.tile([C, 2], F32, name=f"me2{tag}")
                nc.vector.tensor_reduce(me2[:, 0:1], v[:, 1:2, :], AX.X, ALU.add)
                nc.vector.tensor_reduce(me2[:, 1:2], v[:, 4:5, :], AX.X, ALU.add)
                t3 = stats.tile([C, 1], F32, name=f"t3{tag}")
                nc.vector.tensor_tensor(t3[:], me2[:, 0:1], me2[:, 1:2], ALU.add)
                nc.vector.tensor_scalar(Sl[:, 0:1], t3[:], float(CH // 2),
                                        None, ALU.mult)
                m2s = stats.tile([C, 2], F32, name=f"m2s{tag}")
                nc.vector.tensor_reduce(m2s[:, 0:1], v[:, 2:3, :], AX.X, ALU.add)
                nc.vector.tensor_reduce(m2s[:, 1:2], v[:, 5:6, :], AX.X, ALU.add)
                sqm = stats.tile([C, 64], F32, name=f"sqm{tag}")
                nc.scalar.activation(
                    sqm[:, 0:32].rearrange("p (a k) -> p a k", a=1),
                    v[:, 1:2, :], AF.Square)
                nc.scalar.activation(
                    sqm[:, 32:64].rearrange("p (a k) -> p a k", a=1),
                    v[:, 4:5, :], AF.Square)
                sqr = stats.tile([C, 2], F32, name=f"sqr{tag}")
                nc.vector.tensor_reduce(sqr[:, 0:1], sqm[:], AX.X, ALU.add)
                nc.vector.tensor_tensor(sqr[:, 1:2], m2s[:, 0:1], m2s[:, 1:2],
                                        ALU.add)
                t2 = stats.tile([C, 1], F32, name=f"t2{tag}")
                nc.vector.tensor_scalar(t2[:], sqr[:, 0:1], float(CH // 2),
                                        None, ALU.mult)
                nc.vector.tensor_tensor(Sl[:, 1:2], sqr[:, 1:2], t2[:],
                                        ALU.add)
                agin = dramp.tile([C, 2], F32, name=f"agin{tag}")
                nc.gpsimd.dma_start(agin[:], Sl[:])
                agout = dramp.tile([NCORES * C, 2], F32, name=f"agout{tag}")
                nc.gpsimd.collective_compute(
                    "AllGather", ALU.bypass,
                    replica_groups=[list(range(NCORES))],
                    ins=[agin.opt()], outs=[agout.opt()])
                gath = stats.tile([C, 16], F32, name=f"gath{tag}")
                nc.gpsimd.dma_start(
                    gath[:].rearrange("p (r k) -> p r k", r=NCORES),
                    agout[:].rearrange("(r p) k -> p r k", r=NCORES, p=C))
                Sg = stats.tile([C, 2], F32, name=f"Sg{tag}")
                nc.vector.tensor_reduce(
                    Sg[:], gath[:].rearrange("p (r k) -> p k r", r=NCORES),
                    AX.X, ALU.add)
                mv = stats.tile([C, 2], F32, name=f"mv{tag}")
                nc.vector.tensor_scalar(mv[:], Sg[:], 1.0 / N_BN, None, ALU.mult)
                m2 = stats.tile([C, 1], F32, name=f"m2{tag}")
                nc.vector.tensor_tensor(m2[:], mv[:, 0:1], mv[:, 0:1], ALU.mult)
                var = stats.tile([C, 1], F32, name=f"var{tag}")
                nc.vector.tensor_tensor(var[:], mv[:, 1:2], m2[:], ALU.subtract)
                std = stats.tile([C, 1], F32, name=f"std{tag}")
                nc.scalar.activation(std[:], var[:], AF.Sqrt, bias=eps_t[:, 0:1])
                rstd = stats.tile([C, 1], F32, name=f"rstd{tag}")
                nc.vector.reciprocal(rstd[:], std[:])
                a = stats.tile([C, 1], F32, name=f"a{tag}")
                nc.vector.tensor_tensor(a[:], gcol, rstd[:], ALU.mult)
                t1 = stats.tile([C, 1], F32, name=f"t1{tag}")
                nc.vector.tensor_tensor(t1[:], mv[:, 0:1], a[:], ALU.mult)
                sh = stats.tile([C, 1], F32, name=f"sh{tag}")
                nc.vector.tensor_tensor(sh[:], bcol, t1[:], ALU.subtract)
                return a, sh

            # ============ BN apply + residual ============
            def bn_apply(y, a, sh, src, dst, gather_out):
                # dst is either guarded grid tiles (gather_out=False) or the
                # flat 624-wide gather buffer aliased into g0 (gather_out=True)
                for img in range(BL):
                    for q in range(2):
                        n = 286 if (gather_out and q == 1) else CH
                        nr = n // G
                        tmp = work.tile([C, CH], F32R, name="tmp", tag="tmp")
                        nc.scalar.activation(tmp[:, 0:n],
                                             y[img][:, q * CH:q * CH + n],
                                             AF.Relu, bias=sh[:, 0:1],
                                             scale=a[:, 0:1])
                        srcap = gview(src[img])[:, 13 * q:13 * q + nr, 0:G]
                        if gather_out:
                            oap = dst[img][:, q * CH:q * CH + n].rearrange(
                                "p (r c) -> p r c", r=nr, c=G)
                        else:
                            oap = gview(dst[img])[:, 13 * q:13 * q + nr, 0:G]
                        nc.vector.tensor_tensor(
                            oap, srcap,
                            tmp[:, 0:n].rearrange("p (r c) -> p r c",
                                                  r=nr, c=G),
                            ALU.add)

            # ================= Stage B/C/D =================
            conv(g0, wt0, 1, sqs0)
            a0, sh0 = bn_sync(sqs0, bv[:, 1:2], bv[:, 2:3], "0")
            bn_apply(yb, a0, sh0, g0, g1, False)
            conv(g1, wt1, 2, sqs1)
            a1, sh1 = bn_sync(sqs1, bv[:, 3:4], bv[:, 4:5], "1")
            # g2 (gather result) aliases into g0[:, 0:624]
            bn_apply(yb, a1, sh1, g1, g0, True)

            # ================= Stage E: proj_out + LN =================
            SZ = [128, 128, 128, 128, 112]
            for i in range(BL):
                zs = []
                for t in range(5):
                    sz = SZ[t]
                    r0 = i * S + t * 128
                    xr = work.tile([C, H], F32, name="xr", tag="xr", bufs=8)
                    nc.sync.dma_start(xr[0:sz, :], xR[r0:r0 + sz, :])
                    pe = psum.tile([C, CH], F32, tag="ps", name=f"pe{i}_{t}")
                    nc.tensor.matmul(pe[0:sz, 0:H],
                                     g0[i][:, t * 128:t * 128 + sz], wo[:],
                                     start=True, stop=True)
                    z = work.tile([C, H], F32, name="z", tag="z", bufs=8)
                    nc.vector.tensor_tensor(z[0:sz, :], pe[0:sz, 0:H],
                                            xr[0:sz, :], ALU.add)
                    bn6 = work.tile([C, 6], F32, name="bn6", tag="bn6", bufs=8)
                    nc.vector.bn_stats(bn6[0:sz, :], z[0:sz, :])
                    nc.vector.bn_aggr(zst[i][0:sz, 2 * t:2 * t + 2],
                                      bn6[0:sz, :])
                    zs.append(z)
                std5 = work.tile([C, 5], F32, name="std5", tag="std5", bufs=2)
                nc.scalar.activation(
                    std5[:].rearrange("p (a t) -> p a t", a=1),
                    zst[i][:].rearrange("p (t k) -> p k t", k=2)[:, 1:2, :],
                    AF.Sqrt, bias=eps_t[:, 0:1])
                rstd5 = work.tile([C, 5], F32, name="rstd5", tag="rstd5", bufs=2)
                nc.vector.reciprocal(rstd5[:], std5[:])
                for t in range(5):
                    sz = SZ[t]
                    r0 = i * S + t * 128
                    osb = work.tile([C, H], F32, name="osb", tag="osb", bufs=6)
                    nc.vector.tensor_scalar(osb[0:sz, :], zs[t][0:sz, :],
                                            zst[i][0:sz, 2 * t:2 * t + 1],
                                            rstd5[0:sz, t:t + 1],
                                            ALU.subtract, ALU.mult)
                    if ln_affine:
                        o2 = work.tile([C, H], F32, name="o2", tag="o2", bufs=4)
                        nc.vector.tensor_tensor(o2[0:sz, :], osb[0:sz, :],
                                                lng_t[0:sz, :], ALU.mult)
                        nc.gpsimd.tensor_tensor(osb[0:sz, :], o2[0:sz, :],
                                                lnb_t[0:sz, :], ALU.add)
                    if t % 2 == 0:
                        nc.scalar.dma_start(out[r0:r0 + sz, :], osb[0:sz, :])
                    else:
                        nc.sync.dma_start(out[r0:r0 + sz, :], osb[0:sz, :])

    nc.compile()
    return nc


def _get_nc(ln_affine):
    key = ("nc", ln_affine)
    if key not in _CACHE:
        _CACHE[key] = _build(ln_affine)
    return _CACHE[key]


def kernel(x, stab_rows, stab_cols, W_in, b_in,
           conv_w0, conv_b0, bn_g0, bn_b0,
           conv_w1, conv_b1, bn_g1, bn_b1,
           W_out, b_out, ln_g, ln_b, *, _trace=False):
    from concourse.bass_utils import run_bass_kernel_spmd

    x = np.asarray(x, dtype=np.float32)
    W_in = np.asarray(W_in, dtype=np.float32)
    b_in = np.asarray(b_in, dtype=np.float32)
    conv_w0 = np.asarray(conv_w0, dtype=np.float32)
    conv_w1 = np.asarray(conv_w1, dtype=np.float32)
    bn_g0 = np.asarray(bn_g0, dtype=np.float32)
    bn_b0 = np.asarray(bn_b0, dtype=np.float32)
    bn_g1 = np.asarray(bn_g1, dtype=np.float32)
    bn_b1 = np.asarray(bn_b1, dtype=np.float32)
    W_out = np.asarray(W_out, dtype=np.float32)
    b_out = np.asarray(b_out, dtype=np.float32)
    ln_g = np.asarray(ln_g, dtype=np.float32)
    ln_b = np.asarray(ln_b, dtype=np.float32)
    # conv_b0/conv_b1 are no-ops through training-mode BN (shift-invariant).

    ln_affine = not (np.all(ln_g == 1.0) and np.all(ln_b == 0.0))
    nc = _get_nc(ln_affine)

    w0t = np.ascontiguousarray(conv_w0.transpose(2, 3, 1, 0)).reshape(9, C, C)
    w1t = np.ascontiguousarray(conv_w1.transpose(2, 3, 1, 0)).reshape(9, C, C)
    bvec = np.zeros((C, 8), dtype=np.float32)
    bvec[:, 0] = b_in
    bvec[:, 1] = bn_g0
    bvec[:, 2] = bn_b0
    bvec[:, 3] = bn_g1
    bvec[:, 4] = bn_b1

    in_maps = []
    for k in range(NCORES):
        xs = x[k * BL:(k + 1) * BL]
        m = {
            "xT": np.ascontiguousarray(xs.transpose(2, 0, 1)).reshape(H, SL),
            "xR": np.ascontiguousarray((xs + b_out[None, None, :])
                                       .reshape(SL, H)),
            "w_in": W_in,
            "w0": w0t,
            "w1": w1t,
            "w_out": W_out,
            "bvec": bvec,
        }
        if ln_affine:
            m["lng"] = np.ascontiguousarray(
                np.broadcast_to(ln_g[None, :], (C, H)))
            m["lnb"] = np.ascontiguousarray(
                np.broadcast_to(ln_b[None, :], (C, H)))
        in_maps.append(m)

    res = run_bass_kernel_spmd(nc, in_maps, core_ids=list(range(NCORES)),
                               trace=_trace)
    global LAST_EXEC_NS
    LAST_EXEC_NS = res.exec_time_ns
    outs = [res.results[k]["out"] for k in range(NCORES)]
    return np.concatenate(outs, axis=0).reshape(B, S, H)


LAST_EXEC_NS = None



# revision 11
# speedup vs baseline: 1.0245x; 1.0245x over previous
"""Trainium2 Bass kernel for nn_CNNBlock (proj_in -> scatter -> 2x dilated
conv+syncBN+relu+residual -> gather -> proj_out -> residual -> LayerNorm).

Data-parallel over the batch on 8 NeuronCores; BN batch stats synchronized
with a tiny AllGather per conv layer.

Self-contained: hardcodes shapes from the problem spec.
"""
import numpy as np

B = 128          # batch
NCORES = 8
BL = B // NCORES  # 16 images per core
S = 624          # stabilizers = 24*26 grid cells, row-major
H = 256          # hidden
C = 128          # conv channels
G = 26           # grid size
CH = 338         # conv output chunk = 13 rows * 26 cols
HALF = 312       # proj_in chunk = 12 rows * 26 cols
SL = BL * S      # 9984 rows per core
EPS = 1e-5
N_BN = B * G * G  # BN stat count per channel (full batch)
# Guarded grid layout: each of the 26 grid rows is 28 wide (cols 26,27 are
# zero guards) plus a 2-element leading zero guard, so every conv tap reads a
# 26-wide (even) window and out-of-row accesses land on zeros.
GW = 28
GOFF = 2
GBUF = GOFF + GW * G + 2  # 732

_CACHE = {}


def _build(ln_affine):
    import concourse.bacc as bacc
    import concourse.tile as tile
    from concourse import mybir

    F32 = mybir.dt.float32
    F32R = mybir.dt.float32r
    BF16 = mybir.dt.bfloat16
    AF = mybir.ActivationFunctionType
    ALU = mybir.AluOpType
    AX = mybir.AxisListType

    nc = bacc.Bacc("TRN2", target_bir_lowering=False, debug=False,
                   enable_asserts=True, num_devices=NCORES)

    def din(name, shape, dt=F32):
        return nc.dram_tensor(name, shape, dt, kind="ExternalInput").ap()

    xT = din("xT", [H, SL], BF16)    # x shard transposed: [h, img*624+s]
    xR = din("xR", [SL, H])          # x shard + b_out, s-major
    w_in = din("w_in", [H, C], BF16)
    w0 = din("w0", [9, C, C], BF16)  # conv0 weights [tap][c_in][c_out]
    w1 = din("w1", [9, C, C], BF16)
    w_out = din("w_out", [C, H], BF16)
    bvec = din("bvec", [C, 8])       # cols: b_in, bn_g0, bn_b0, bn_g1, bn_b1
    if ln_affine:
        lng = din("lng", [C, H])     # ln_g broadcast over partitions
        lnb = din("lnb", [C, H])
    out = nc.dram_tensor("out", [SL, H], F32, kind="ExternalOutput").ap()

    with tile.TileContext(nc) as tc:
        with (
            tc.tile_pool(name="const", bufs=1) as const,
            tc.tile_pool(name="grids", bufs=1) as grids,
            tc.tile_pool(name="stream", bufs=6) as stream,
            tc.tile_pool(name="work", bufs=3) as work,
            tc.tile_pool(name="stats", bufs=1) as stats,
            tc.tile_pool(name="psum", bufs=8, space="PSUM") as psum,
            tc.tile_pool(name="dramp", bufs=4, space="DRAM") as dramp,
        ):
            # ---- constants ----
            bv = const.tile([C, 8], F32, name="bv")
            nc.sync.dma_start(bv[:], bvec[:, :])
            wi = []
            for k in range(2):
                w = const.tile([C, C], BF16, name=f"wi{k}")
                nc.sync.dma_start(w[:], w_in[k * C:(k + 1) * C, :])
                wi.append(w)
            if ln_affine:
                lng_t = const.tile([C, H], F32, name="lng_t")
                nc.sync.dma_start(lng_t[:], lng[:, :])
                lnb_t = const.tile([C, H], F32, name="lnb_t")
                nc.sync.dma_start(lnb_t[:], lnb[:, :])
            zf = const.tile([C, 64], F32, name="zf")
            nc.vector.memset(zf[:], 0.0)
            zr = const.tile([C, 64], BF16, name="zr")
            nc.vector.tensor_copy(zr[:], zf[:])

            # Startup barrier: a tiny AllGather issued first absorbs the
            # cross-core launch skew while the front of the kernel (DMAs,
            # proj_in) runs, so the first real sync-BN AllGather doesn't
            # pay it. Collectives execute in order on the TOPSP queue.
            bar_in = dramp.tile([C, 2], F32, name="bar_in")
            nc.gpsimd.dma_start(bar_in[:], zf[:, 0:2])
            bar_out = dramp.tile([NCORES * C, 2], F32, name="bar_out")
            nc.gpsimd.collective_compute(
                "AllGather", ALU.bypass,
                replica_groups=[list(range(NCORES))],
                ins=[bar_in.opt()], outs=[bar_out.opt()])
            bar_sb = const.tile([C, 2], F32, name="bar_sb")
            nc.gpsimd.dma_start(bar_sb[:], bar_out[0:C, :])
            # eps_t = barrier*0 + EPS ties the barrier into the dataflow
            eps_t = const.tile([C, 1], F32, name="eps_t")
            nc.vector.tensor_scalar(eps_t[:], bar_sb[:, 0:1], 0.0, EPS,
                                    ALU.mult, ALU.add)

            # ---- persistent per-image buffers ----
            g0 = [grids.tile([C, GBUF], BF16, name=f"g0_{i}") for i in range(BL)]
            g1 = [grids.tile([C, GBUF], BF16, name=f"g1_{i}") for i in range(BL)]
            yb = [grids.tile([C, G * G], BF16, name=f"y_{i}") for i in range(BL)]

            def gview(t):
                # [C, 26, 28] row view of a guarded grid buffer
                return t[:, GOFF:GOFF + G * GW].rearrange(
                    "p (r c) -> p r c", r=G, c=GW)

            # zero the guard elements (they persist for the whole kernel)
            for i in range(BL):
                for t, empty_rows in ((g0[i], True), (g1[i], False)):
                    nc.vector.tensor_copy(t[:, 0:GOFF], zr[:, 0:GOFF])
                    nc.vector.tensor_copy(
                        gview(t)[:, :, G:GW],
                        zr[:, 0:2 * G].rearrange("p (r c) -> p r c", r=G, c=2))
                    if empty_rows:
                        # grid rows 24-25 hold no stabilizers -> zeros
                        nc.vector.tensor_copy(
                            t[:, GOFF + 24 * GW:GOFF + 26 * GW],
                            zr[:, 0:2 * GW])

            sqs0 = stats.tile([C, 192], F32, name="sqs0")
            sqs1 = stats.tile([C, 192], F32, name="sqs1")
            zst = [stats.tile([C, 10], F32, name=f"zst_{i}") for i in range(BL)]
            for i in range(BL):
                nc.vector.memset(zst[i][:], 1.0)

            # ================= Stage A: proj_in =================
            for i in range(BL):
                for q in range(2):
                    o = i * S + q * HALF
                    xk0 = stream.tile([C, HALF], BF16, name="xk0", tag="xk0")
                    nc.sync.dma_start(xk0[:], xT[0:C, o:o + HALF])
                    xk1 = stream.tile([C, HALF], BF16, name="xk1", tag="xk1")
                    nc.sync.dma_start(xk1[:], xT[C:H, o:o + HALF])
                    pa = psum.tile([C, CH], F32, tag="ps", name=f"pa{i}_{q}")
                    nc.tensor.matmul(pa[:, 0:HALF], wi[0][:], xk0[:],
                                     start=True, stop=False)
                    nc.tensor.matmul(pa[:, 0:HALF], wi[1][:], xk1[:],
                                     start=False, stop=True)
                    # 312 = 12 grid rows of 26, rows 12q..12q+11
                    dst = gview(g0[i])[:, 12 * q:12 * q + 12, 0:G]
                    nc.scalar.activation(
                        dst, pa[:, 0:HALF].rearrange("p (r c) -> p r c",
                                                     r=12, c=G),
                        AF.Identity, bias=bv[:, 0:1], scale=1.0)

            # deferred weight loads: keep the startup DMA queues free for
            # the proj_in activation stream
            wt0, wt1 = [], []
            for t in range(9):
                w = const.tile([C, C], BF16, name=f"w0_{t}")
                nc.sync.dma_start(w[:], w0[t, :, :])
                wt0.append(w)
            for t in range(9):
                w = const.tile([C, C], BF16, name=f"w1_{t}")
                nc.sync.dma_start(w[:], w1[t, :, :])
                wt1.append(w)
            wo = const.tile([C, H], BF16, name="wo")
            nc.sync.dma_start(wo[:], w_out[:, :])

            # ================= conv + stats helper =================
            def conv(src, wt, dil, sqs):
                # groups of 4 chunks (2 images): only half the PSUM banks per
                # group, so the next group's matmuls stream while this group's
                # PSUM->SBUF copies drain (keeps the PE HAM-warm)
                for grp in range(8):
                    psc = [psum.tile([C, CH], F32, tag="ps",
                                     name=f"pc{dil}_{grp}_{m}") for m in range(4)]
                    for t9 in range(9):
                        di = (t9 // 3 - 1) * dil
                        dj = (t9 % 3 - 1) * dil
                        for m in range(4):
                            img = grp * 2 + m // 2
                            q = m % 2
                            # clip rows so i+di stays in [0, 26); cols always
                            # span the full 26 (out-of-row cols hit guards=0)
                            r_lo = max(13 * q, -di)
                            r_hi = min(13 * q + 13, G - di)
                            nr = r_hi - r_lo
                            base = GOFF + (r_lo + di) * GW + dj
                            rhs = src[img][:, base:base + nr * GW].rearrange(
                                "p (r c) -> p r c", r=nr, c=GW)[:, :, 0:G]
                            oap = psc[m][:, (r_lo - 13 * q) * G:
                                          (r_hi - 13 * q) * G]
                            nc.tensor.matmul(oap, wt[t9][:], rhs,
                                             start=(t9 == 0), stop=(t9 == 8))
                    for m in range(4):
                        img = grp * 2 + m // 2
                        q = m % 2
                        cid = img * 2 + q
                        ysl = yb[img][:, q * CH:(q + 1) * CH]
                        nc.vector.tensor_copy(ysl, psc[m][:])
                        # per-chunk (count, mean, M2) even/odd triplets; both
                        # the sum and sumsq are reassembled in bn_sync
                        nc.vector.bn_stats(sqs[:, cid * 6:(cid + 1) * 6], ysl)

            # ============ sync-BN stats: AllGather + combine ============
            def bn_sync(sqs, gcol, bcol, tag):
                Sl = stats.tile([C, 2], F32, name=f"Sl{tag}")
                # sum = 169 * sum(mean_e + mean_o) over the 32 chunks;
                # sumsq = sum(M2_e + M2_o) + 169 * sum(mean_e^2 + mean_o^2)
                v = sqs[:].rearrange("p (k t) -> p t k", k=32, t=6)
                me2 = stats.tile([C, 2], F32, name=f"me2{tag}")
                nc.vector.tensor_reduce(me2[:, 0:1], v[:, 1:2, :], AX.X, ALU.add)
                nc.vector.tensor_reduce(me2[:, 1:2], v[:, 4:5, :], AX.X, ALU.add)
                t3 = stats.tile([C, 1], F32, name=f"t3{tag}")
                nc.vector.tensor_tensor(t3[:], me2[:, 0:1], me2[:, 1:2], ALU.add)
                nc.vector.tensor_scalar(Sl[:, 0:1], t3[:], float(CH // 2),
                                        None, ALU.mult)
                m2s = stats.tile([C, 2], F32, name=f"m2s{tag}")
                nc.vector.tensor_reduce(m2s[:, 0:1], v[:, 2:3, :], AX.X, ALU.add)
                nc.vector.tensor_reduce(m2s[:, 1:2], v[:, 5:6, :], AX.X, ALU.add)
                sqm = stats.tile([C, 64], F32, name=f"sqm{tag}")
                nc.scalar.activation(
                    sqm[:, 0:32].rearrange("p (a k) -> p a k", a=1),
                    v[:, 1:2, :], AF.Square)
                nc.scalar.activation(
                    sqm[:, 32:64].rearrange("p (a k) -> p a k", a=1),
                    v[:, 4:5, :], AF.Square)
                sqr = stats.tile([C, 2], F32, name=f"sqr{tag}")
                nc.vector.tensor_reduce(sqr[:, 0:1], sqm[:], AX.X, ALU.add)
                nc.vector.tensor_tensor(sqr[:, 1:2], m2s[:, 0:1], m2s[:, 1:2],
                                        ALU.add)
                t2 = stats.tile([C, 1], F32, name=f"t2{tag}")
                nc.vector.tensor_scalar(t2[:], sqr[:, 0:1], float(CH // 2),
                                        None, ALU.mult)
                nc.vector.tensor_tensor(Sl[:, 1:2], sqr[:, 1:2], t2[:],
                                        ALU.add)
                agin = dramp.tile([C, 2], F32, name=f"agin{tag}")
                nc.gpsimd.dma_start(agin[:], Sl[:])
                agout = dramp.tile([NCORES * C, 2], F32, name=f"agout{tag}")
                nc.gpsimd.collective_compute(
                    "AllGather", ALU.bypass,
                    replica_groups=[list(range(NCORES))],
                    ins=[agin.opt()], outs=[agout.opt()])
                gath = stats.tile([C, 16], F32, name=f"gath{tag}")
                nc.gpsimd.dma_start(
                    gath[:].rearrange("p (r k) -> p r k", r=NCORES),
                    agout[:].rearrange("(r p) k -> p r k", r=NCORES, p=C))
                Sg = stats.tile([C, 2], F32, name=f"Sg{tag}")
                nc.vector.tensor_reduce(
                    Sg[:], gath[:].rearrange("p (r k) -> p k r", r=NCORES),
                    AX.X, ALU.add)
                mv = stats.tile([C, 2], F32, name=f"mv{tag}")
                nc.vector.tensor_scalar(mv[:], Sg[:], 1.0 / N_BN, None, ALU.mult)
                m2 = stats.tile([C, 1], F32, name=f"m2{tag}")
                nc.vector.tensor_tensor(m2[:], mv[:, 0:1], mv[:, 0:1], ALU.mult)
                var = stats.tile([C, 1], F32, name=f"var{tag}")
                nc.vector.tensor_tensor(var[:], mv[:, 1:2], m2[:], ALU.subtract)
                std = stats.tile([C, 1], F32, name=f"std{tag}")
                nc.scalar.activation(std[:], var[:], AF.Sqrt, bias=eps_t[:, 0:1])
                rstd = stats.tile([C, 1], F32, name=f"rstd{tag}")
                nc.vector.reciprocal(rstd[:], std[:])
                a = stats.tile([C, 1], F32, name=f"a{tag}")
                nc.vector.tensor_tensor(a[:], gcol, rstd[:], ALU.mult)
                t1 = stats.tile([C, 1], F32, name=f"t1{tag}")
                nc.vector.tensor_tensor(t1[:], mv[:, 0:1], a[:], ALU.mult)
                sh = stats.tile([C, 1], F32, name=f"sh{tag}")
                nc.vector.tensor_tensor(sh[:], bcol, t1[:], ALU.subtract)
                return a, sh

            # ============ BN apply + residual ============
            def bn_apply(y, a, sh, src, dst, gather_out):
                # dst is either guarded grid tiles (gather_out=False) or the
                # flat 624-wide gather buffer aliased into g0 (gather_out=True)
                for img in range(BL):
                    for q in range(2):
                        n = 286 if (gather_out and q == 1) else CH
                        nr = n // G
                        tmp = work.tile([C, CH], BF16, name="tmp", tag="tmp")
                        nc.scalar.activation(tmp[:, 0:n],
                                             y[img][:, q * CH:q * CH + n],
                                             AF.Relu, bias=sh[:, 0:1],
                                             scale=a[:, 0:1])
                        srcap = gview(src[img])[:, 13 * q:13 * q + nr, 0:G]
                        if gather_out:
                            oap = dst[img][:, q * CH:q * CH + n].rearrange(
                                "p (r c) -> p r c", r=nr, c=G)
                        else:
                            oap = gview(dst[img])[:, 13 * q:13 * q + nr, 0:G]
                        nc.vector.tensor_tensor(
                            oap, srcap,
                            tmp[:, 0:n].rearrange("p (r c) -> p r c",
                                                  r=nr, c=G),
                            ALU.add)

            # ================= Stage B/C/D =================
            conv(g0, wt0, 1, sqs0)
            a0, sh0 = bn_sync(sqs0, bv[:, 1:2], bv[:, 2:3], "0")
            bn_apply(yb, a0, sh0, g0, g1, False)
            conv(g1, wt1, 2, sqs1)
            a1, sh1 = bn_sync(sqs1, bv[:, 3:4], bv[:, 4:5], "1")
            # g2 (gather result) aliases into g0[:, 0:624]
            bn_apply(yb, a1, sh1, g1, g0, True)

            # ================= Stage E: proj_out + LN =================
            SZ = [128, 128, 128, 128, 112]
            for i in range(BL):
                zs = []
                for t in range(5):
                    sz = SZ[t]
                    r0 = i * S + t * 128
                    xr = work.tile([C, H], F32, name="xr", tag="xr", bufs=8)
                    nc.sync.dma_start(xr[0:sz, :], xR[r0:r0 + sz, :])
                    pe = psum.tile([C, CH], F32, tag="ps", name=f"pe{i}_{t}")
                    nc.tensor.matmul(pe[0:sz, 0:H],
                                     g0[i][:, t * 128:t * 128 + sz], wo[:],
                                     start=True, stop=True)
                    z = work.tile([C, H], F32, name="z", tag="z", bufs=8)
                    nc.vector.tensor_tensor(z[0:sz, :], pe[0:sz, 0:H],
                                            xr[0:sz, :], ALU.add)
                    bn6 = work.tile([C, 6], F32, name="bn6", tag="bn6", bufs=8)
                    nc.vector.bn_stats(bn6[0:sz, :], z[0:sz, :])
                    nc.vector.bn_aggr(zst[i][0:sz, 2 * t:2 * t + 2],
                                      bn6[0:sz, :])
                    zs.append(z)
                std5 = work.tile([C, 5], F32, name="std5", tag="std5", bufs=2)
                nc.scalar.activation(
                    std5[:].rearrange("p (a t) -> p a t", a=1),
                    zst[i][:].rearrange("p (t k) -> p k t", k=2)[:, 1:2, :],
                    AF.Sqrt, bias=eps_t[:, 0:1])
                rstd5 = work.tile([C, 5], F32, name="rstd5", tag="rstd5", bufs=2)
                nc.vector.reciprocal(rstd5[:], std5[:])
                for t in range(5):
                    sz = SZ[t]
                    r0 = i * S + t * 128
                    osb = work.tile([C, H], F32, name="osb", tag="osb", bufs=6)
                    nc.vector.tensor_scalar(osb[0:sz, :], zs[t][0:sz, :],
                                            zst[i][0:sz, 2 * t:2 * t + 1],
                                            rstd5[0:sz, t:t + 1],
                                            ALU.subtract, ALU.mult)
                    if ln_affine:
                        o2 = work.tile([C, H], F32, name="o2", tag="o2", bufs=4)
                        nc.vector.tensor_tensor(o2[0:sz, :], osb[0:sz, :],
                                                lng_t[0:sz, :], ALU.mult)
                        nc.gpsimd.tensor_tensor(osb[0:sz, :], o2[0:sz, :],
                                                lnb_t[0:sz, :], ALU.add)
                    if t % 2 == 0:
                        nc.scalar.dma_start(out[r0:r0 + sz, :], osb[0:sz, :])
                    else:
                        nc.sync.dma_start(out[r0:r0 + sz, :], osb[0:sz, :])

    nc.compile()
    return nc


def _get_nc(ln_affine):
    key = ("nc", ln_affine)
    if key not in _CACHE:
        _CACHE[key] = _build(ln_affine)
    return _CACHE[key]


def kernel(x, stab_rows, stab_cols, W_in, b_in,
           conv_w0, conv_b0, bn_g0, bn_b0,
           conv_w1, conv_b1, bn_g1, bn_b1,
           W_out, b_out, ln_g, ln_b, *, _trace=False):
    from concourse.bass_utils import run_bass_kernel_spmd
    from ml_dtypes import bfloat16

    x = np.asarray(x, dtype=np.float32)
    W_in = np.asarray(W_in, dtype=np.float32)
    b_in = np.asarray(b_in, dtype=np.float32)
    conv_w0 = np.asarray(conv_w0, dtype=np.float32)
    conv_w1 = np.asarray(conv_w1, dtype=np.float32)
    bn_g0 = np.asarray(bn_g0, dtype=np.float32)
    bn_b0 = np.asarray(bn_b0, dtype=np.float32)
    bn_g1 = np.asarray(bn_g1, dtype=np.float32)
    bn_b1 = np.asarray(bn_b1, dtype=np.float32)
    W_out = np.asarray(W_out, dtype=np.float32)
    b_out = np.asarray(b_out, dtype=np.float32)
    ln_g = np.asarray(ln_g, dtype=np.float32)
    ln_b = np.asarray(ln_b, dtype=np.float32)
    # conv_b0/conv_b1 are no-ops through training-mode BN (shift-invariant).

    ln_affine = not (np.all(ln_g == 1.0) and np.all(ln_b == 0.0))
    nc = _get_nc(ln_affine)

    w0t = np.ascontiguousarray(conv_w0.transpose(2, 3, 1, 0)).reshape(9, C, C) \
        .astype(bfloat16)
    w1t = np.ascontiguousarray(conv_w1.transpose(2, 3, 1, 0)).reshape(9, C, C) \
        .astype(bfloat16)
    bvec = np.zeros((C, 8), dtype=np.float32)
    bvec[:, 0] = b_in
    bvec[:, 1] = bn_g0
    bvec[:, 2] = bn_b0
    bvec[:, 3] = bn_g1
    bvec[:, 4] = bn_b1

    in_maps = []
    for k in range(NCORES):
        xs = x[k * BL:(k + 1) * BL]
        m = {
            "xT": np.ascontiguousarray(xs.transpose(2, 0, 1)).reshape(H, SL)
                  .astype(bfloat16),
            "xR": np.ascontiguousarray((xs + b_out[None, None, :])
                                       .reshape(SL, H)),
            "w_in": W_in.astype(bfloat16),
            "w0": w0t,
            "w1": w1t,
            "w_out": W_out.astype(bfloat16),
            "bvec": bvec,
        }
        if ln_affine:
            m["lng"] = np.ascontiguousarray(
                np.broadcast_to(ln_g[None, :], (C, H)))
            m["lnb"] = np.ascontiguousarray(
                np.broadcast_to(ln_b[None, :], (C, H)))
        in_maps.append(m)

    res = run_bass_kernel_spmd(nc, in_maps, core_ids=list(range(NCORES)),
                               trace=_trace)
    global LAST_EXEC_NS
    LAST_EXEC_NS = res.exec_time_ns
    outs = [res.results[k]["out"] for k in range(NCORES)]
    return np.concatenate(outs, axis=0).reshape(B, S, H)


LAST_EXEC_NS = None

